# revision 12
# baseline (speedup 1.0000x reference)
"""AdaConv2d distributed Bass kernel for 8 TRN2 NeuronCores.

Reference computation:
  x [4,512,128,128] -> instance_norm -> per-sample grouped 3x3 conv (128 groups,
  4->4) -> grouped 1x1 conv (4->1) + bias -> concat to [1,512,128,128] ->
  dense 3x3 conv 512->512 (reflect pad) + bias -> [1,512,128,128]

Decomposition (validated vs reference in numpy):
  * grouped 3x3 + grouped 1x1 fuse into one grouped 3x3 conv with
    weff[b,g,u,:,:] = sum_v wp[b,g,v] * ws[b,g*4+v,u,:,:]
  * instance norm folds into stage-1 weights: w2 = weff * inv[cin],
    bias2[g] = -sum_{u,tap} weff[g,u,tap] * mu[g,u] * inv[g,u]
    so x is consumed raw (bf16) by the matmuls; only per-channel
    sum / sumsq stats are needed (16 KB AllReduce across the 8 cores).

Sharding: core r owns output rows [16r, 16r+16). It receives a 20-row x slab
(reflect-gathered rows, reflect-padded cols, width 130) and computes an
18-row ys slab = ys_pad rows [16r, 16r+18). Boundary cores fix the one
duplicated reflect row via a data-driven blend (uniform SPMD graph); the
column reflect of ys is two in-SBUF column copies (uniform for all cores).

Stage-1 matmuls: per cin-block cb, lhsT is [K=128, M=32] (only 32 group
outputs per block) placed at PSUM partitions 32cb..32cb+32 via
tile_position=(0,32cb) -> the 4 blocks run concurrently in the PE array.
Stage-2: dense shift-and-matmul, 36 accumulating bf16 matmuls per psum chunk.
"""
import numpy as np
import ml_dtypes

import concourse.bass as bass
import concourse.bacc as bacc
import concourse.tile as tile
import concourse.mybir as mybir
from concourse.bass_utils import run_bass_kernel_spmd

F32 = mybir.dt.float32
BF16 = mybir.dt.bfloat16
AOT = mybir.AluOpType
AXT = mybir.AxisListType

B = 4
G = 128
H = 128
W = 128
NCORES = 8
ROWS = H // NCORES          # 16 output rows per core
SLAB = ROWS + 2             # 18 ys slab rows (= ys_pad rows 16r..16r+18)
XR = SLAB + 2               # 20 x slab rows
XC = W + 2                  # 130 x slab cols (reflect-padded)
EPS = 1e-5
CHUNKS1 = [(0, 4), (4, 4), (8, 4), (12, 4), (16, 2)]   # ys slab row chunks
CHUNKS2 = [(0, 4), (4, 4), (8, 4), (12, 4)]            # output row chunks


def build_nc():
    nc = bacc.Bacc(num_devices=NCORES)

    xs = nc.dram_tensor("xs", [B, 4, 128, XR, XC], BF16, kind="ExternalInput")
    wst = nc.dram_tensor("wst", [B, 4, 128, 4, 9], F32, kind="ExternalInput")
    wpt = nc.dram_tensor("wpt", [B, 4, 128, 4], F32, kind="ExternalInput")
    bi = nc.dram_tensor("bi", [B, 128], F32, kind="ExternalInput")
    cwt = nc.dram_tensor("cwt", [128, 9, 4, 512], BF16, kind="ExternalInput")
    cbv = nc.dram_tensor("cbv", [4, 128], F32, kind="ExternalInput")
    e32 = nc.dram_tensor("e32", [128, 32], F32, kind="ExternalInput")
    fxv = nc.dram_tensor("fxv", [128, 4], F32, kind="ExternalInput")
    out = nc.dram_tensor("out", [4, 128, ROWS, W], F32, kind="ExternalOutput")

    with tile.TileContext(nc) as tc:
        with (
            tc.tile_pool(name="xp", bufs=1) as xp,
            tc.tile_pool(name="wp", bufs=1) as wp,
            tc.tile_pool(name="yp", bufs=1) as yp,
            tc.tile_pool(name="sp", bufs=2) as sp,
            tc.tile_pool(name="ps", bufs=1, space="PSUM") as psp,
            tc.tile_pool(name="psx", bufs=1, space="PSUM") as psbp,
            tc.tile_pool(name="dr", bufs=1, space="DRAM") as dr,
        ):
            # persistent tiles
            xt = [[xp.tile([128, XR, XC], BF16, tag=f"x{b}{cb}", name=f"x{b}{cb}")
                   for cb in range(4)] for b in range(B)]
            yt = [yp.tile([128, SLAB, XC], BF16, tag=f"y{b}", name=f"y{b}")
                  for b in range(B)]
            cwt_sb = wp.tile([128, 9, 4, 512], BF16, tag="cwt", name="cwt")
            cwsum = wp.tile([128, 4, 512], BF16, tag="cwsum", name="cwsum")
            wst_sb = wp.tile([128, 16, 4, 9], F32, tag="wst", name="wst")
            wpt_sb = wp.tile([128, 16, 4], F32, tag="wpt", name="wpt")
            bi_sb = wp.tile([128, B], F32, tag="bi", name="bi")
            cb_sb = wp.tile([128, 4], F32, tag="cb", name="cb")
            e32_sb = wp.tile([128, 32], F32, tag="e32", name="e32")
            fx_sb = wp.tile([128, 4], F32, tag="fx", name="fx")
            eps_sb = wp.tile([128, 1], F32, tag="eps", name="eps")
            stat = wp.tile([128, 32], F32, tag="stat", name="stat")
            mvall = wp.tile([128, 16, 2], F32, tag="mvall", name="mvall")
            m2h = wp.tile([128, 16], F32, tag="m2h", name="m2h")
            stat8 = wp.tile([128, 8, 32], F32, tag="stat8", name="stat8")
            stat4 = wp.tile([128, 4, 32], F32, tag="stat4", name="stat4")
            stat2 = wp.tile([128, 2, 32], F32, tag="stat2", name="stat2")
            statg = wp.tile([128, 32], F32, tag="statg", name="statg")
            mean = wp.tile([128, 16], F32, tag="mean", name="mean")
            ex2 = wp.tile([128, 16], F32, tag="ex2", name="ex2")
            m2 = wp.tile([128, 16], F32, tag="m2", name="m2")
            var = wp.tile([128, 16], F32, tag="var", name="var")
            sd = wp.tile([128, 16], F32, tag="sd", name="sd")
            inv = wp.tile([128, 16], F32, tag="inv", name="inv")
            weff = wp.tile([128, 16, 9], F32, tag="weff", name="weff")
            wtmp = wp.tile([128, 16, 9], F32, tag="wtmp", name="wtmp")
            w2 = wp.tile([128, 16, 9], F32, tag="w2", name="w2")
            w2m_s = wp.tile([128, 16, 9], F32, tag="w2ms", name="w2ms")
            w2m = wp.tile([128, 16], F32, tag="w2m", name="w2m")
            lhs1 = wp.tile([128, 16, 9, 32], BF16, tag="lhs1", name="lhs1")
            btot = wp.tile([128, B], F32, tag="btot", name="btot")
            btot_h = wp.tile([128, B], BF16, tag="btot_h", name="btot_h")
            bias_eff = wp.tile([128, 4], F32, tag="bias_eff", name="bias_eff")

            cc_in = dr.tile([128, 32], F32, tag="ccin", name="ccin")
            cc_out = dr.tile([NCORES, 128, 32], F32, tag="ccout", name="ccout")

            nc.vector.memset(eps_sb[:], EPS)

            # ---- head: x interior DMAs + partial stats + PE warm-up ----
            with nc.named_scope("head"):
                for b in range(B):
                    for cb in range(4):
                        idx = b * 4 + cb
                        nc.sync.dma_start(out=xt[b][cb][:, 2:2 + ROWS, :],
                                          in_=xs[b, cb, :, 2:2 + ROWS, :])
                        sl = xt[b][cb][:, 2:2 + ROWS, 1:1 + W]
                        nc.vector.reduce_sum(out=stat[:, idx:idx + 1], in_=sl,
                                             axis=AXT.XY)
                        if idx < 11:
                            sq = sp.tile([128, ROWS, W], F32, tag="sq", name="sq")
                            nc.scalar.activation(
                                out=sq[:], in_=sl,
                                func=mybir.ActivationFunctionType.Square,
                                accum_out=stat[:, 16 + idx:17 + idx])
                        else:
                            sq = sp.tile([128, ROWS, W], BF16, tag="sqv",
                                         name="sqv")
                            nc.vector.tensor_mul(sq[:], sl, sl)
                            nc.vector.reduce_sum(out=stat[:, 16 + idx:17 + idx],
                                                 in_=sq[:], axis=AXT.XY)
                        # PE warm-up: junk matmuls paced by the DMA stream
                        pw = psbp.tile([128, 512], F32, tag="warm", name="warm")
                        for _ in range(3):
                            nc.tensor.matmul(
                                pw[:], lhsT=xt[b][cb][:, 2, 0:128],
                                rhs=xt[b][cb][:, 3:7, 1:129],
                                start=True, stop=True)
                # halo rows + weights
                for b in range(B):
                    for cb in range(4):
                        nc.sync.dma_start(out=xt[b][cb][:, 0:2, :],
                                          in_=xs[b, cb, :, 0:2, :])
                        nc.sync.dma_start(out=xt[b][cb][:, 2 + ROWS:XR, :],
                                          in_=xs[b, cb, :, 2 + ROWS:XR, :])
                for b in range(B):
                    for cb in range(4):
                        idx = b * 4 + cb
                        nc.sync.dma_start(out=wst_sb[:, idx, :, :], in_=wst[b, cb])
                        nc.sync.dma_start(out=wpt_sb[:, idx, :], in_=wpt[b, cb])
                    nc.sync.dma_start(out=bi_sb[:, b:b + 1], in_=bi[b, :, None])
                for ob in range(4):
                    nc.sync.dma_start(out=cb_sb[:, ob:ob + 1], in_=cbv[ob, :, None])
                nc.sync.dma_start(out=e32_sb[:], in_=e32[:, :])
                nc.sync.dma_start(out=fx_sb[:], in_=fxv[:, :])
                nc.sync.dma_start(out=cwt_sb[:], in_=cwt[:, :, :, :])
                # cwsum[:, cbb, :] = sum_t cwt[:, t, cbb, :]  (pre-AR filler work)
                for cbb in range(4):
                    nc.vector.tensor_add(cwsum[:, cbb, :], cwt_sb[:, 0, cbb, :],
                                         cwt_sb[:, 1, cbb, :])
                    for t in range(2, 9):
                        nc.vector.tensor_add(cwsum[:, cbb, :], cwsum[:, cbb, :],
                                             cwt_sb[:, t, cbb, :])

            # ---- stats AllGather + local sum ----
            with nc.named_scope("ar"):
                nc.gpsimd.dma_start(out=cc_in[:], in_=stat[:])
                nc.gpsimd.collective_compute(
                    "AllGather", AOT.bypass,
                    replica_groups=[list(range(NCORES))],
                    ins=[cc_in[:].opt()], outs=[cc_out[:].opt()])
                nc.gpsimd.dma_start(
                    out=stat8[:], in_=cc_out[:, :, :].rearrange("r p f -> p r f"))
                nc.vector.tensor_add(stat4[:], stat8[:, 0:4, :], stat8[:, 4:8, :])
                nc.vector.tensor_add(stat2[:], stat4[:, 0:2, :], stat4[:, 2:4, :])
                nc.vector.tensor_add(statg[:], stat2[:, 0, :], stat2[:, 1, :])

            # ---- stat math + stage-1 weight prep ----
            with nc.named_scope("wprep"):
                cntinv = 1.0 / float(ROWS * W * NCORES)
                nc.vector.tensor_scalar_mul(out=mean[:], in0=statg[:, 0:16],
                                            scalar1=cntinv)
                nc.vector.tensor_scalar_mul(out=ex2[:], in0=statg[:, 16:32],
                                            scalar1=cntinv)
                nc.vector.tensor_mul(m2[:], mean[:], mean[:])
                nc.vector.tensor_sub(var[:], ex2[:], m2[:])
                nc.scalar.activation(out=sd[:], in_=var[:],
                                     func=mybir.ActivationFunctionType.Sqrt,
                                     bias=eps_sb[:, 0:1])
                nc.vector.reciprocal(inv[:], sd[:])

                nc.vector.tensor_tensor(
                    weff[:], wst_sb[:, :, 0, :],
                    wpt_sb[:, :, 0, None].broadcast_to([128, 16, 9]), AOT.mult)
                for v in (1, 2, 3):
                    nc.vector.tensor_tensor(
                        wtmp[:], wst_sb[:, :, v, :],
                        wpt_sb[:, :, v, None].broadcast_to([128, 16, 9]), AOT.mult)
                    nc.vector.tensor_add(weff[:], weff[:], wtmp[:])
                nc.vector.tensor_tensor(
                    w2[:], weff[:], inv[:, :, None].broadcast_to([128, 16, 9]),
                    AOT.mult)
                nc.vector.tensor_tensor(
                    lhs1[:],
                    e32_sb[:, None, None, :].broadcast_to([128, 16, 9, 32]),
                    w2[:, :, :, None].broadcast_to([128, 16, 9, 32]), AOT.mult)
                nc.vector.tensor_tensor(
                    w2m_s[:], w2[:], mean[:, :, None].broadcast_to([128, 16, 9]),
                    AOT.mult)
                nc.vector.reduce_sum(out=w2m[:], in_=w2m_s[:], axis=AXT.X)

                # btot = bi - E32^T @ w2m ; bias_eff = conv_b + cwsum^T @ btot
                psb = psbp.tile([128, B], F32, tag="psb", name="psb")
                for b in range(B):
                    for cb in range(4):
                        idx = b * 4 + cb
                        nc.tensor.matmul(
                            psb[32 * cb:32 * cb + 32, b:b + 1],
                            lhsT=e32_sb[:, :], rhs=w2m[:, idx:idx + 1],
                            start=True, stop=True, tile_position=(0, 32 * cb))
                nc.vector.tensor_sub(btot[:], bi_sb[:], psb[:])
                nc.vector.tensor_copy(btot_h[:], btot[:])
                psbe = psbp.tile([128, 4], F32, tag="psbe", name="psbe")
                for ob in range(4):
                    for cbb in range(4):
                        nc.tensor.matmul(
                            psbe[:, ob:ob + 1],
                            lhsT=cwsum[:, cbb, 128 * ob:128 * (ob + 1)],
                            rhs=btot_h[:, cbb:cbb + 1],
                            start=(cbb == 0), stop=(cbb == 3))
                nc.vector.tensor_add(bias_eff[:], cb_sb[:], psbe[:])

            # ---- stage 1: ys slabs (no bias; folded into stage 2) ----
            with nc.named_scope("s1"):
                for b in range(B):
                    pss = [psp.tile([128, 4, 128], F32, tag=f"c{c}",
                                    name=f"ps1_{c}") for c in range(4)]
                    for t in range(9):
                        ky, kx = divmod(t, 3)
                        for cb in range(4):
                            idx = b * 4 + cb
                            for c in range(4):
                                r0 = 4 * c
                                nc.tensor.matmul(
                                    pss[c][32 * cb:32 * cb + 32, :, :],
                                    lhsT=lhs1[:, idx, t, :],
                                    rhs=xt[b][cb][:, r0 + ky:r0 + ky + 4, kx:kx + W],
                                    start=(t == 0), stop=(t == 8),
                                    tile_position=(0, 32 * cb))
                    for c in range(4):
                        nc.scalar.copy(yt[b][:, 4 * c:4 * c + 4, 1:1 + W],
                                       pss[c][:, :, :])
                    # last 2 rows (16..18)
                    ps4 = psp.tile([128, 4, 128], F32, tag="c4", name="ps1_4")
                    for t in range(9):
                        ky, kx = divmod(t, 3)
                        for cb in range(4):
                            idx = b * 4 + cb
                            nc.tensor.matmul(
                                ps4[32 * cb:32 * cb + 32, 0:2, :],
                                lhsT=lhs1[:, idx, t, :],
                                rhs=xt[b][cb][:, 16 + ky:18 + ky, kx:kx + W],
                                start=(t == 0), stop=(t == 8),
                                tile_position=(0, 32 * cb))
                    nc.scalar.copy(yt[b][:, 16:18, 1:1 + W], ps4[:, 0:2, :])

                # fixups: row blend (boundary cores) + column reflect
                for b in range(B):
                    tmp0 = sp.tile([128, XC], BF16, tag="fixtmp", name="fixtmp")
                    nc.vector.tensor_scalar(
                        out=tmp0[:], in0=yt[b][:, 2, :], scalar1=fx_sb[:, 1:2],
                        scalar2=None, op0=AOT.mult)
                    nc.vector.scalar_tensor_tensor(
                        out=yt[b][:, 0, :], in0=yt[b][:, 0, :],
                        scalar=fx_sb[:, 0:1], in1=tmp0[:],
                        op0=AOT.mult, op1=AOT.add)
                    tmp1 = sp.tile([128, XC], BF16, tag="fixtmp", name="fixtmp1")
                    nc.vector.tensor_scalar(
                        out=tmp1[:], in0=yt[b][:, SLAB - 3, :],
                        scalar1=fx_sb[:, 3:4], scalar2=None, op0=AOT.mult)
                    nc.vector.scalar_tensor_tensor(
                        out=yt[b][:, SLAB - 1, :], in0=yt[b][:, SLAB - 1, :],
                        scalar=fx_sb[:, 2:3], in1=tmp1[:],
                        op0=AOT.mult, op1=AOT.add)
                    nc.vector.tensor_copy(yt[b][:, :, 0:1], yt[b][:, :, 2:3])
                    nc.vector.tensor_copy(yt[b][:, :, XC - 1:XC],
                                          yt[b][:, :, XC - 3:XC - 2])

            # ---- stage 2: dense conv + bias_eff -> out ----
            with nc.named_scope("s2"):
                for ob in range(4):
                    pss = [psp.tile([128, 4, 128], F32, tag=f"c{c}",
                                    name=f"ps2_{c}") for c in range(4)]
                    k = 0
                    for t in range(9):
                        ky, kx = divmod(t, 3)
                        for cbb in range(4):
                            lhsT = cwt_sb[:, t, cbb, 128 * ob:128 * (ob + 1)]
                            for c in range(4):
                                t0 = 4 * c
                                nc.tensor.matmul(
                                    pss[c][:, :, :], lhsT=lhsT,
                                    rhs=yt[cbb][:, t0 + ky:t0 + ky + 4, kx:kx + W],
                                    start=(t == 0 and cbb == 0),
                                    stop=(t == 8 and cbb == 3))
                    for c in range(4):
                        osb = sp.tile([128, 4, 128], F32, tag="osb", name="osb")
                        nc.scalar.add(osb[:], pss[c][:, :, :],
                                      bias_eff[:, ob:ob + 1])
                        nc.sync.dma_start(out=out[ob, :, 4 * c:4 * c + 4, :],
                                          in_=osb[:])

    nc.compile()
    return nc


_CACHE = {}


def _get_nc():
    if "nc" not in _CACHE:
        _CACHE["nc"] = build_nc()
    return _CACHE["nc"]


def _prepare_in_maps(inputs):
    x = np.ascontiguousarray(np.asarray(inputs["x"], np.float32))
    ws = np.asarray(inputs["w_spatial"], np.float32)
    wp = np.asarray(inputs["w_pointwise"], np.float32)
    bias = np.asarray(inputs["bias"], np.float32)
    cw = np.asarray(inputs["conv_w"], np.float32)
    cbv = np.asarray(inputs["conv_b"], np.float32)
    bf16 = ml_dtypes.bfloat16

    # columns reflect-padded once; rows gathered per core
    xpadc = np.pad(x, ((0, 0), (0, 0), (0, 0), (1, 1)), mode="reflect")

    # wst[b, cb, (gl,u), v, tap] from ws[b, (g,v), u, ky, kx]
    ws_r = ws.reshape(B, G, 4, 4, 3, 3)                      # b, g, v, u, ky, kx
    wst_h = ws_r.transpose(0, 1, 3, 2, 4, 5).reshape(B, G, 4, 4, 9)
    wst_h = np.ascontiguousarray(wst_h.reshape(B, 4, 32, 4, 4, 9)
                                 .reshape(B, 4, 128, 4, 9)).astype(np.float32)
    # wpt[b, cb, (gl,u), v] = wp[b, g, v] replicated over u
    wp_ = wp[:, :, :, 0, 0]                                  # b, g, v
    wpt_h = np.broadcast_to(wp_[:, :, None, :], (B, G, 4, 4))
    wpt_h = np.ascontiguousarray(wpt_h.reshape(B, 4, 32, 4, 4)
                                 .reshape(B, 4, 128, 4)).astype(np.float32)
    bi_h = np.ascontiguousarray(bias).astype(np.float32)     # [B, 128]
    # cwt[c_local, tap, cbb, cout] from cw[cout, cin, ky, kx]
    t1 = cw.transpose(1, 2, 3, 0).reshape(4, 128, 9, 512)    # cbb, c_local, tap, cout
    cwt_h = np.ascontiguousarray(t1.transpose(1, 2, 0, 3)).astype(bf16)
    cbv_h = np.ascontiguousarray(cbv.reshape(4, 128)).astype(np.float32)
    e32_h = np.zeros((128, 32), np.float32)
    e32_h[np.arange(128), np.arange(128) // 4] = 1.0

    in_maps = []
    for r in range(NCORES):
        rows = np.arange(16 * r - 2, 16 * r + 18)
        rows = np.where(rows < 0, -rows, rows)
        rows = np.where(rows >= H, 2 * H - 2 - rows, rows)
        xs_h = np.ascontiguousarray(
            xpadc[:, :, rows, :].reshape(B, 4, 128, XR, XC)).astype(bf16)
        lo = 0.0 if r == 0 else 1.0
        hi = 0.0 if r == NCORES - 1 else 1.0
        fx = np.array([lo, 1.0 - lo, hi, 1.0 - hi], np.float32)
        fxv_h = np.ascontiguousarray(np.tile(fx, (128, 1)))
        in_maps.append({
            "xs": xs_h, "wst": wst_h, "wpt": wpt_h, "bi": bi_h,
            "cwt": cwt_h, "cbv": cbv_h, "e32": e32_h, "fxv": fxv_h,
        })
    return in_maps


def _assemble(results):
    parts = [np.asarray(results[r]["out"], np.float32).reshape(512, ROWS, W)
             for r in range(NCORES)]
    return np.concatenate(parts, axis=1)[None]


def run(inputs, **kwargs):
    in_maps = _prepare_in_maps(inputs)
    res = run_bass_kernel_spmd(_get_nc(), in_maps, core_ids=list(range(NCORES)),
                               **kwargs)
    return _assemble(res.results), res


def kernel(**inputs):
    out, _ = run(inputs)
    return out


# revision 13
# speedup vs baseline: 1.0461x; 1.0461x over previous
"""AdaConv2d distributed Bass kernel for 8 TRN2 NeuronCores.

Reference computation:
  x [4,512,128,128] -> instance_norm -> per-sample grouped 3x3 conv (128 groups,
  4->4) -> grouped 1x1 conv (4->1) + bias -> concat to [1,512,128,128] ->
  dense 3x3 conv 512->512 (reflect pad) + bias -> [1,512,128,128]

Decomposition (validated vs reference in numpy):
  * grouped 3x3 + grouped 1x1 fuse into one grouped 3x3 conv with
    weff[b,g,u,:,:] = sum_v wp[b,g,v] * ws[b,g*4+v,u,:,:]
  * instance norm folds into stage-1 weights: w2 = weff * inv[cin],
    bias2[g] = -sum_{u,tap} weff[g,u,tap] * mu[g,u] * inv[g,u]
    so x is consumed raw (bf16) by the matmuls; only per-channel
    sum / sumsq stats are needed (16 KB AllReduce across the 8 cores).

Sharding: core r owns output rows [16r, 16r+16). It receives a 20-row x slab
(reflect-gathered rows, reflect-padded cols, width 130) and computes an
18-row ys slab = ys_pad rows [16r, 16r+18). Boundary cores fix the one
duplicated reflect row via a data-driven blend (uniform SPMD graph); the
column reflect of ys is two in-SBUF column copies (uniform for all cores).

Stage-1 matmuls: per cin-block cb, lhsT is [K=128, M=32] (only 32 group
outputs per block) placed at PSUM partitions 32cb..32cb+32 via
tile_position=(0,32cb) -> the 4 blocks run concurrently in the PE array.
Stage-2: dense shift-and-matmul, 36 accumulating bf16 matmuls per psum chunk.
"""
import numpy as np
import ml_dtypes

import concourse.bass as bass
import concourse.bacc as bacc
import concourse.tile as tile
import concourse.mybir as mybir
from concourse.bass_utils import run_bass_kernel_spmd

F32 = mybir.dt.float32
BF16 = mybir.dt.bfloat16
AOT = mybir.AluOpType
AXT = mybir.AxisListType

B = 4
G = 128
H = 128
W = 128
NCORES = 8
ROWS = H // NCORES          # 16 output rows per core
SLAB = ROWS + 2             # 18 ys slab rows (= ys_pad rows 16r..16r+18)
XR = SLAB + 2               # 20 x slab rows
XC = W + 2                  # 130 x slab cols (reflect-padded)
EPS = 1e-5
CHUNKS1 = [(0, 4), (4, 4), (8, 4), (12, 4), (16, 2)]   # ys slab row chunks
CHUNKS2 = [(0, 4), (4, 4), (8, 4), (12, 4)]            # output row chunks


def build_nc():
    nc = bacc.Bacc(num_devices=NCORES)

    xs = nc.dram_tensor("xs", [B, 4, 128, XR, XC], BF16, kind="ExternalInput")
    wst = nc.dram_tensor("wst", [B, 4, 128, 4, 9], F32, kind="ExternalInput")
    wpt = nc.dram_tensor("wpt", [B, 4, 128, 4], F32, kind="ExternalInput")
    bi = nc.dram_tensor("bi", [B, 128], F32, kind="ExternalInput")
    cwt = nc.dram_tensor("cwt", [128, 9, 4, 512], BF16, kind="ExternalInput")
    cbv = nc.dram_tensor("cbv", [4, 128], F32, kind="ExternalInput")
    e32 = nc.dram_tensor("e32", [128, 32], F32, kind="ExternalInput")
    fxv = nc.dram_tensor("fxv", [128, 4], F32, kind="ExternalInput")
    out = nc.dram_tensor("out", [4, 128, ROWS, W], F32, kind="ExternalOutput")

    with tile.TileContext(nc) as tc:
        with (
            tc.tile_pool(name="xp", bufs=1) as xp,
            tc.tile_pool(name="wp", bufs=1) as wp,
            tc.tile_pool(name="yp", bufs=1) as yp,
            tc.tile_pool(name="sp", bufs=2) as sp,
            tc.tile_pool(name="ps", bufs=1, space="PSUM") as psp,
            tc.tile_pool(name="psx", bufs=1, space="PSUM") as psbp,
            tc.tile_pool(name="dr", bufs=1, space="DRAM") as dr,
        ):
            # persistent tiles
            xt = [[xp.tile([128, XR, XC], BF16, tag=f"x{b}{cb}", name=f"x{b}{cb}")
                   for cb in range(4)] for b in range(B)]
            yt = [yp.tile([128, SLAB, XC], BF16, tag=f"y{b}", name=f"y{b}")
                  for b in range(B)]
            cwt_sb = wp.tile([128, 9, 4, 512], BF16, tag="cwt", name="cwt")
            cwsum = wp.tile([128, 4, 512], BF16, tag="cwsum", name="cwsum")
            wst_sb = wp.tile([128, 16, 4, 9], F32, tag="wst", name="wst")
            wpt_sb = wp.tile([128, 16, 4], F32, tag="wpt", name="wpt")
            bi_sb = wp.tile([128, B], F32, tag="bi", name="bi")
            cb_sb = wp.tile([128, 4], F32, tag="cb", name="cb")
            e32_sb = wp.tile([128, 32], F32, tag="e32", name="e32")
            fx_sb = wp.tile([128, 4], F32, tag="fx", name="fx")
            eps_sb = wp.tile([128, 1], F32, tag="eps", name="eps")
            stat = wp.tile([128, 32], F32, tag="stat", name="stat")
            mvall = wp.tile([128, 16, 2], F32, tag="mvall", name="mvall")
            m2h = wp.tile([128, 16], F32, tag="m2h", name="m2h")
            stat8 = wp.tile([128, 8, 32], F32, tag="stat8", name="stat8")
            stat4 = wp.tile([128, 4, 32], F32, tag="stat4", name="stat4")
            stat2 = wp.tile([128, 2, 32], F32, tag="stat2", name="stat2")
            statg = wp.tile([128, 32], F32, tag="statg", name="statg")
            mean = wp.tile([128, 16], F32, tag="mean", name="mean")
            ex2 = wp.tile([128, 16], F32, tag="ex2", name="ex2")
            m2 = wp.tile([128, 16], F32, tag="m2", name="m2")
            var = wp.tile([128, 16], F32, tag="var", name="var")
            sd = wp.tile([128, 16], F32, tag="sd", name="sd")
            inv = wp.tile([128, 16], F32, tag="inv", name="inv")
            weff = wp.tile([128, 16, 9], F32, tag="weff", name="weff")
            wtmp = wp.tile([128, 16, 9], F32, tag="wtmp", name="wtmp")
            w2 = wp.tile([128, 16, 9], F32, tag="w2", name="w2")
            w2m_s = wp.tile([128, 16, 9], F32, tag="w2ms", name="w2ms")
            w2m = wp.tile([128, 16], F32, tag="w2m", name="w2m")
            lhs1 = wp.tile([128, 16, 9, 32], BF16, tag="lhs1", name="lhs1")
            btot = wp.tile([128, B], F32, tag="btot", name="btot")
            btot_h = wp.tile([128, B], BF16, tag="btot_h", name="btot_h")
            bias_eff = wp.tile([128, 4], F32, tag="bias_eff", name="bias_eff")

            cc_in = dr.tile([128, 32], F32, tag="ccin", name="ccin")
            cc_out = dr.tile([NCORES, 128, 32], F32, tag="ccout", name="ccout")

            nc.vector.memset(eps_sb[:], EPS)

            # ---- head: x interior DMAs + partial stats + PE warm-up ----
            with nc.named_scope("head"):
                for b in range(B):
                    for cb in range(4):
                        idx = b * 4 + cb
                        nc.sync.dma_start(out=xt[b][cb][:, 2:2 + ROWS, :],
                                          in_=xs[b, cb, :, 2:2 + ROWS, :])
                        sl = xt[b][cb][:, 2:2 + ROWS, 1:1 + W]
                        nc.vector.reduce_sum(out=stat[:, idx:idx + 1], in_=sl,
                                             axis=AXT.XY)
                        sq = sp.tile([128, ROWS, W], F32, tag="sq", name="sq")
                        nc.scalar.activation(
                            out=sq[:], in_=sl,
                            func=mybir.ActivationFunctionType.Square,
                            accum_out=stat[:, 16 + idx:17 + idx])
                        # PE warm-up: junk matmuls paced by the DMA stream
                        pw = psbp.tile([128, 512], F32, tag="warm", name="warm")
                        for _ in range(3):
                            nc.tensor.matmul(
                                pw[:], lhsT=xt[b][cb][:, 2, 0:128],
                                rhs=xt[b][cb][:, 3:7, 1:129],
                                start=True, stop=True)
                # halo rows + weights
                for b in range(B):
                    for cb in range(4):
                        nc.sync.dma_start(out=xt[b][cb][:, 0:2, :],
                                          in_=xs[b, cb, :, 0:2, :])
                        nc.sync.dma_start(out=xt[b][cb][:, 2 + ROWS:XR, :],
                                          in_=xs[b, cb, :, 2 + ROWS:XR, :])
                for b in range(B):
                    for cb in range(4):
                        idx = b * 4 + cb
                        nc.sync.dma_start(out=wst_sb[:, idx, :, :], in_=wst[b, cb])
                        nc.sync.dma_start(out=wpt_sb[:, idx, :], in_=wpt[b, cb])
                    nc.sync.dma_start(out=bi_sb[:, b:b + 1], in_=bi[b, :, None])
                for ob in range(4):
                    nc.sync.dma_start(out=cb_sb[:, ob:ob + 1], in_=cbv[ob, :, None])
                nc.sync.dma_start(out=e32_sb[:], in_=e32[:, :])
                nc.sync.dma_start(out=fx_sb[:], in_=fxv[:, :])
                nc.sync.dma_start(out=cwt_sb[:], in_=cwt[:, :, :, :])
                # cwsum[:, cbb, :] = sum_t cwt[:, t, cbb, :]  (pre-AR filler work)
                for cbb in range(4):
                    nc.vector.tensor_add(cwsum[:, cbb, :], cwt_sb[:, 0, cbb, :],
                                         cwt_sb[:, 1, cbb, :])
                    for t in range(2, 9):
                        nc.vector.tensor_add(cwsum[:, cbb, :], cwsum[:, cbb, :],
                                             cwt_sb[:, t, cbb, :])

            # ---- stats AllGather + local sum ----
            with nc.named_scope("ar"):
                nc.gpsimd.dma_start(out=cc_in[:], in_=stat[:])
                nc.gpsimd.collective_compute(
                    "AllGather", AOT.bypass,
                    replica_groups=[list(range(NCORES))],
                    ins=[cc_in[:].opt()], outs=[cc_out[:].opt()])
                nc.gpsimd.dma_start(
                    out=stat8[:], in_=cc_out[:, :, :].rearrange("r p f -> p r f"))
                nc.vector.tensor_add(stat4[:], stat8[:, 0:4, :], stat8[:, 4:8, :])
                nc.vector.tensor_add(stat2[:], stat4[:, 0:2, :], stat4[:, 2:4, :])
                nc.vector.tensor_add(statg[:], stat2[:, 0, :], stat2[:, 1, :])

            # ---- stat math + stage-1 weight prep ----
            with nc.named_scope("wprep"):
                cntinv = 1.0 / float(ROWS * W * NCORES)
                nc.vector.tensor_scalar_mul(out=mean[:], in0=statg[:, 0:16],
                                            scalar1=cntinv)
                nc.vector.tensor_scalar_mul(out=ex2[:], in0=statg[:, 16:32],
                                            scalar1=cntinv)
                nc.vector.tensor_mul(m2[:], mean[:], mean[:])
                nc.vector.tensor_sub(var[:], ex2[:], m2[:])
                nc.scalar.activation(out=sd[:], in_=var[:],
                                     func=mybir.ActivationFunctionType.Sqrt,
                                     bias=eps_sb[:, 0:1])
                nc.vector.reciprocal(inv[:], sd[:])

                nc.vector.tensor_tensor(
                    weff[:], wst_sb[:, :, 0, :],
                    wpt_sb[:, :, 0, None].broadcast_to([128, 16, 9]), AOT.mult)
                for v in (1, 2, 3):
                    nc.vector.tensor_tensor(
                        wtmp[:], wst_sb[:, :, v, :],
                        wpt_sb[:, :, v, None].broadcast_to([128, 16, 9]), AOT.mult)
                    nc.vector.tensor_add(weff[:], weff[:], wtmp[:])
                nc.vector.tensor_tensor(
                    w2[:], weff[:], inv[:, :, None].broadcast_to([128, 16, 9]),
                    AOT.mult)
                nc.vector.tensor_tensor(
                    lhs1[:],
                    e32_sb[:, None, None, :].broadcast_to([128, 16, 9, 32]),
                    w2[:, :, :, None].broadcast_to([128, 16, 9, 32]), AOT.mult)
                nc.vector.tensor_tensor(
                    w2m_s[:], w2[:], mean[:, :, None].broadcast_to([128, 16, 9]),
                    AOT.mult)
                nc.vector.reduce_sum(out=w2m[:], in_=w2m_s[:], axis=AXT.X)

                # btot = bi - E32^T @ w2m ; bias_eff = conv_b + cwsum^T @ btot
                psb = psbp.tile([128, B], F32, tag="psb", name="psb")
                for b in range(B):
                    for cb in range(4):
                        idx = b * 4 + cb
                        nc.tensor.matmul(
                            psb[32 * cb:32 * cb + 32, b:b + 1],
                            lhsT=e32_sb[:, :], rhs=w2m[:, idx:idx + 1],
                            start=True, stop=True, tile_position=(0, 32 * cb))
                nc.vector.tensor_sub(btot[:], bi_sb[:], psb[:])
                nc.vector.tensor_copy(btot_h[:], btot[:])
                psbe = psbp.tile([128, 4], F32, tag="psbe", name="psbe")
                for ob in range(4):
                    for cbb in range(4):
                        nc.tensor.matmul(
                            psbe[:, ob:ob + 1],
                            lhsT=cwsum[:, cbb, 128 * ob:128 * (ob + 1)],
                            rhs=btot_h[:, cbb:cbb + 1],
                            start=(cbb == 0), stop=(cbb == 3))
                nc.vector.tensor_add(bias_eff[:], cb_sb[:], psbe[:])

            # ---- stage 1: ys slabs (no bias; folded into stage 2) ----
            with nc.named_scope("s1"):
                for b in range(B):
                    pss = [psp.tile([128, 4, 128], F32, tag=f"c{c}",
                                    name=f"ps1_{c}") for c in range(4)]
                    for t in range(9):
                        ky, kx = divmod(t, 3)
                        for cb in range(4):
                            idx = b * 4 + cb
                            for c in range(4):
                                r0 = 4 * c
                                nc.tensor.matmul(
                                    pss[c][32 * cb:32 * cb + 32, :, :],
                                    lhsT=lhs1[:, idx, t, :],
                                    rhs=xt[b][cb][:, r0 + ky:r0 + ky + 4, kx:kx + W],
                                    start=(t == 0), stop=(t == 8),
                                    tile_position=(0, 32 * cb))
                    for c in range(4):
                        nc.scalar.copy(yt[b][:, 4 * c:4 * c + 4, 1:1 + W],
                                       pss[c][:, :, :])
                    # last 2 rows (16..18)
                    ps4 = psp.tile([128, 4, 128], F32, tag="c4", name="ps1_4")
                    for t in range(9):
                        ky, kx = divmod(t, 3)
                        for cb in range(4):
                            idx = b * 4 + cb
                            nc.tensor.matmul(
                                ps4[32 * cb:32 * cb + 32, 0:2, :],
                                lhsT=lhs1[:, idx, t, :],
                                rhs=xt[b][cb][:, 16 + ky:18 + ky, kx:kx + W],
                                start=(t == 0), stop=(t == 8),
                                tile_position=(0, 32 * cb))
                    nc.scalar.copy(yt[b][:, 16:18, 1:1 + W], ps4[:, 0:2, :])

                # fixups: row blend (boundary cores) + column reflect
                for b in range(B):
                    tmp0 = sp.tile([128, XC], BF16, tag="fixtmp", name="fixtmp")
                    nc.vector.tensor_scalar(
                        out=tmp0[:], in0=yt[b][:, 2, :], scalar1=fx_sb[:, 1:2],
                        scalar2=None, op0=AOT.mult)
                    nc.vector.scalar_tensor_tensor(
                        out=yt[b][:, 0, :], in0=yt[b][:, 0, :],
                        scalar=fx_sb[:, 0:1], in1=tmp0[:],
                        op0=AOT.mult, op1=AOT.add)
                    tmp1 = sp.tile([128, XC], BF16, tag="fixtmp", name="fixtmp1")
                    nc.vector.tensor_scalar(
                        out=tmp1[:], in0=yt[b][:, SLAB - 3, :],
                        scalar1=fx_sb[:, 3:4], scalar2=None, op0=AOT.mult)
                    nc.vector.scalar_tensor_tensor(
                        out=yt[b][:, SLAB - 1, :], in0=yt[b][:, SLAB - 1, :],
                        scalar=fx_sb[:, 2:3], in1=tmp1[:],
                        op0=AOT.mult, op1=AOT.add)
                    nc.vector.tensor_copy(yt[b][:, :, 0:1], yt[b][:, :, 2:3])
                    nc.vector.tensor_copy(yt[b][:, :, XC - 1:XC],
                                          yt[b][:, :, XC - 3:XC - 2])

            # ---- stage 2: dense conv + bias_eff -> out ----
            with nc.named_scope("s2"):
                for ob in range(4):
                    pss = [psp.tile([128, 4, 128], F32, tag=f"c{c}",
                                    name=f"ps2_{c}") for c in range(4)]
                    k = 0
                    for t in range(9):
                        ky, kx = divmod(t, 3)
                        for cbb in range(4):
                            lhsT = cwt_sb[:, t, cbb, 128 * ob:128 * (ob + 1)]
                            for c in range(4):
                                t0 = 4 * c
                                nc.tensor.matmul(
                                    pss[c][:, :, :], lhsT=lhsT,
                                    rhs=yt[cbb][:, t0 + ky:t0 + ky + 4, kx:kx + W],
                                    start=(t == 0 and cbb == 0),
                                    stop=(t == 8 and cbb == 3))
                    for c in range(4):
                        osb = sp.tile([128, 4, 128], F32, tag="osb", name="osb")
                        nc.scalar.add(osb[:], pss[c][:, :, :],
                                      bias_eff[:, ob:ob + 1])
                        nc.sync.dma_start(out=out[ob, :, 4 * c:4 * c + 4, :],
                                          in_=osb[:])

    nc.compile()
    return nc


_CACHE = {}


def _get_nc():
    if "nc" not in _CACHE:
        _CACHE["nc"] = build_nc()
    return _CACHE["nc"]


def _prepare_in_maps(inputs):
    x = np.ascontiguousarray(np.asarray(inputs["x"], np.float32))
    ws = np.asarray(inputs["w_spatial"], np.float32)
    wp = np.asarray(inputs["w_pointwise"], np.float32)
    bias = np.asarray(inputs["bias"], np.float32)
    cw = np.asarray(inputs["conv_w"], np.float32)
    cbv = np.asarray(inputs["conv_b"], np.float32)
    bf16 = ml_dtypes.bfloat16

    # columns reflect-padded once; rows gathered per core
    xpadc = np.pad(x, ((0, 0), (0, 0), (0, 0), (1, 1)), mode="reflect")

    # wst[b, cb, (gl,u), v, tap] from ws[b, (g,v), u, ky, kx]
    ws_r = ws.reshape(B, G, 4, 4, 3, 3)                      # b, g, v, u, ky, kx
    wst_h = ws_r.transpose(0, 1, 3, 2, 4, 5).reshape(B, G, 4, 4, 9)
    wst_h = np.ascontiguousarray(wst_h.reshape(B, 4, 32, 4, 4, 9)
                                 .reshape(B, 4, 128, 4, 9)).astype(np.float32)
    # wpt[b, cb, (gl,u), v] = wp[b, g, v] replicated over u
    wp_ = wp[:, :, :, 0, 0]                                  # b, g, v
    wpt_h = np.broadcast_to(wp_[:, :, None, :], (B, G, 4, 4))
    wpt_h = np.ascontiguousarray(wpt_h.reshape(B, 4, 32, 4, 4)
                                 .reshape(B, 4, 128, 4)).astype(np.float32)
    bi_h = np.ascontiguousarray(bias).astype(np.float32)     # [B, 128]
    # cwt[c_local, tap, cbb, cout] from cw[cout, cin, ky, kx]
    t1 = cw.transpose(1, 2, 3, 0).reshape(4, 128, 9, 512)    # cbb, c_local, tap, cout
    cwt_h = np.ascontiguousarray(t1.transpose(1, 2, 0, 3)).astype(bf16)
    cbv_h = np.ascontiguousarray(cbv.reshape(4, 128)).astype(np.float32)
    e32_h = np.zeros((128, 32), np.float32)
    e32_h[np.arange(128), np.arange(128) // 4] = 1.0

    in_maps = []
    for r in range(NCORES):
        rows = np.arange(16 * r - 2, 16 * r + 18)
        rows = np.where(rows < 0, -rows, rows)
        rows = np.where(rows >= H, 2 * H - 2 - rows, rows)
        xs_h = np.ascontiguousarray(
            xpadc[:, :, rows, :].reshape(B, 4, 128, XR, XC)).astype(bf16)
        lo = 0.0 if r == 0 else 1.0
        hi = 0.0 if r == NCORES - 1 else 1.0
        fx = np.array([lo, 1.0 - lo, hi, 1.0 - hi], np.float32)
        fxv_h = np.ascontiguousarray(np.tile(fx, (128, 1)))
        in_maps.append({
            "xs": xs_h, "wst": wst_h, "wpt": wpt_h, "bi": bi_h,
            "cwt": cwt_h, "cbv": cbv_h, "e32": e32_h, "fxv": fxv_h,
        })
    return in_maps


def _assemble(results):
    parts = [np.asarray(results[r]["out"], np.float32).reshape(512, ROWS, W)
             for r in range(NCORES)]
    return np.concatenate(parts, axis=1)[None]


def run(inputs, **kwargs):
    in_maps = _prepare_in_maps(inputs)
    res = run_bass_kernel_spmd(_get_nc(), in_maps, core_ids=list(range(NCORES)),
                               **kwargs)
    return _assemble(res.results), res


def kernel(**inputs):
    out, _ = run(inputs)
    return out
